# revision 5
# baseline (speedup 1.0000x reference)
"""DMPNN layer kernel for Trainium2, data-parallel over batch on 8 NeuronCores.

Math (reference):
    gate[i,j]  = (sum_b adj[b,i,j]) > 0                      [N,N], shared across batch
    hW[b,i,o]  = sum_c h[b,i,c] * Wh[o,c]                    Wh = W_w[:, :H]
    term_h     = sum_i gate[i,j] * hW[b,i,o]
    e_sum      = sum_i gate[i,j] * edge_attr[b,i,j,e]
    term_e     = sum_e e_sum[b,j,e] * We[o,e]                We = W_w[:, H:]
    count[j]   = sum_i gate[i,j]
    msg        = term_h + term_e + count[j]*W_b[o]
    msg       *= (j < num_nodes[b])
    h_new      = (h + msg) @ U_w.T + U_b

Design (per core, BL = 4 batches; target_regime = memory):
  - edge_attr is the dominant HBM stream.  It is cast host-side to fp8
    (e4m3, "fp8" variant) or bf16 ("bf16" variant): rel tolerance is 2e-2
    and the edge contribution is diluted through We/U_w, so fp8 costs only
    ~1e-3 output error while halving/quartering HBM traffic vs f32.
  - gate is computed on device from the host bit-packed adj words
    (word[i,j] has bit b set iff adj[b,i,j] != 0  ->  any-over-batch is a
    single != 0 compare; no collective needed since every core reads the
    256 KB word matrix).
  - gating of the edge stream is a bitwise AND on a uint16 view of the
    fp8 pairs with a 0xFFFF/0x0000 mask (exact zeroing, 2x DVE mode), or
    a bf16 multiply by a 0/1 mask for the bf16 variant.
  - the i-reduction of the gated stream runs on the PE: per (batch, e)
    one fp8 DoubleRow matmul contracts both 128-row i-chunks at once
    (sel[k, c, m] = 1[m == e]) into es[e, j]; bf16 variant uses one
    matmul per (chunk, e).
  - everything is kept feature-major ("T" layout, [hidden, nodes]); h and
    the weight transposes are prepared host-side so no on-device
    transposes are needed.  y is written back transposed bf16 and
    un-transposed on host.
  - per-batch stages are software-pipelined with a skew of one batch so
    each in-order engine queue (PE / DVE / Act / Pool / SP-DMA) stays
    busy: AND(b+1) is issued before the msg/up tail of batch b.
"""

import os
import sys

for _p in ("/opt/trn_rl_repo", "/root/.axon_site/_ro/trn_rl_repo"):
    if _p not in sys.path:
        sys.path.insert(0, _p)

import numpy as np

import concourse.bass as bass
import concourse.tile as tile
from concourse import bacc, mybir
from concourse.bass_utils import run_bass_kernel_spmd

B, N, H, E = 32, 256, 128, 16
N_CORES = 8
BL = B // N_CORES          # batches per core
NJE = N * E                # 4096
F32 = mybir.dt.float32
BF16 = mybir.dt.bfloat16
U16 = mybir.dt.uint16
FP8 = mybir.dt.float8e4
I32 = mybir.dt.int32
AOP = mybir.AluOpType


def build_nc(reps: int = 1, variant: str = "fp8"):
    """variant: "fp8"  - edge stream in fp8 e4m3, AND-gating on uint16 view,
                         DoubleRow est matmuls (both i-chunks per matmul)
                "bf16" - edge stream in bf16, multiply-gating, per-chunk est
    """
    fp8 = variant == "fp8"
    EDT = FP8 if fp8 else BF16           # edge dtype
    ECOLS = 2 * NJE                      # 8192 edge elems per partition row
    MCOLS = ECOLS // 2 if fp8 else ECOLS  # u16 AND-mask cols (fp8 pairs)
    MREP = (E // 2) if fp8 else E        # mask replication per j

    nc = bacc.Bacc("TRN2", target_bir_lowering=False, debug=False,
                   num_devices=N_CORES)

    d_ht = nc.dram_tensor("ht", [BL, H, N], BF16, kind="ExternalInput")
    d_ea = nc.dram_tensor("ea", [BL, N, NJE], EDT, kind="ExternalInput")
    # adj bit-packed host-side (lossless): bit b of word [i, j] is
    # adj[b, i, j] != 0; any-over-batch == word != 0.
    d_adj = nc.dram_tensor("adjb", [N, N], I32, kind="ExternalInput")
    d_mask = nc.dram_tensor("mask", [1, BL * N], BF16, kind="ExternalInput")
    d_whT = nc.dram_tensor("whT", [H, H], BF16, kind="ExternalInput")
    d_weT = nc.dram_tensor("weT", [E, H], BF16, kind="ExternalInput")
    d_uwT = nc.dram_tensor("uwT", [H, H], BF16, kind="ExternalInput")
    d_wb = nc.dram_tensor("wb", [1, H], BF16, kind="ExternalInput")
    d_ubc = nc.dram_tensor("ubc", [H, 1], F32, kind="ExternalInput")
    d_ones = nc.dram_tensor("ones", [H, 1], BF16, kind="ExternalInput")
    # DoubleRow selectors: sel[k, 32*e + 16*c + m] = 1[m == e]  (fp8)
    # plain selectors:     sel[k, 16*e + m]        = 1[m == e]  (bf16)
    d_sel = nc.dram_tensor("sel", [128, 32 * E if fp8 else 16 * E], EDT,
                           kind="ExternalInput")
    d_y = nc.dram_tensor("y", [BL, H, N], BF16, kind="ExternalOutput")

    with tile.TileContext(nc) as tc:
        with (
            tc.tile_pool(name="const", bufs=1) as cpool,
            tc.tile_pool(name="gatep", bufs=2) as gpool,
            tc.tile_pool(name="ea", bufs=6) as eapool,
            tc.tile_pool(name="work", bufs=2) as wpool,
            tc.tile_pool(name="ps_es", bufs=2, space="PSUM") as ps_es,
            tc.tile_pool(name="ps_hw", bufs=2, space="PSUM") as ps_hw,
            tc.tile_pool(name="ps_msg", bufs=2, space="PSUM") as ps_msg,
            tc.tile_pool(name="ps_up", bufs=1, space="PSUM") as ps_up,
            tc.tile_pool(name="ps_cnt", bufs=1, space="PSUM") as ps_cnt,
        ):
            # ---- constants (preamble, not per-rep) -------------------------
            whT = cpool.tile([H, H], BF16)
            nc.sync.dma_start(whT[:], d_whT[:])
            weT = cpool.tile([E, H], BF16)
            nc.sync.dma_start(weT[:], d_weT[:])
            uwT = cpool.tile([H, H], BF16)
            nc.sync.dma_start(uwT[:], d_uwT[:])
            wb = cpool.tile([1, H], BF16)
            nc.sync.dma_start(wb[:], d_wb[:])
            ubc = cpool.tile([H, 1], F32)
            nc.sync.dma_start(ubc[:], d_ubc[:])
            ones = cpool.tile([H, 1], BF16)
            nc.sync.dma_start(ones[:], d_ones[:])
            sel = cpool.tile([128, 32 * E if fp8 else 16 * E], EDT)
            nc.sync.dma_start(sel[:], d_sel[:])

            for rep in range(reps):
                # ---- gate from packed adj words ------------------------
                at = gpool.tile([128, 2 * N], I32, name="at")
                nc.sync.dma_start(
                    at[:].rearrange("p (c j) -> p c j", c=2),
                    d_adj[:].rearrange("(c p) j -> p c j", c=2))
                # hT for all 4 batches in one DMA
                hT = gpool.tile([H, BL * N], BF16, name="hT")
                nc.sync.dma_start(
                    hT[:].rearrange("p (b j) -> p b j", b=BL),
                    d_ht[:].rearrange("b p j -> p b j"))
                # node masks, one row DMA, broadcast per batch on Pool
                mrows = gpool.tile([1, BL * N], BF16, name="mrows")
                nc.scalar.dma_start(mrows[:], d_mask[:])
                maskb = []
                for b in range(BL):
                    mb = gpool.tile([128, N], BF16, name=f"maskb{b}")
                    nc.gpsimd.partition_broadcast(mb[:],
                                                  mrows[0:1, bass.ts(b, N)])
                    maskb.append(mb)

                g = []
                for c in range(2):
                    gc = gpool.tile([128, N], BF16, name=f"g{c}")
                    nc.vector.tensor_scalar(gc[:], at[:, bass.ts(c, N)],
                                            0, None, AOP.not_equal)
                    g.append(gc)
                if fp8:
                    m16 = gpool.tile([128, 2 * N], U16, name="m16")
                    nc.vector.tensor_scalar(m16[:], at[:], 0, 65535,
                                            AOP.not_equal, AOP.mult)
                # gating mask over the full (c, j, e) edge row
                mbc = gpool.tile([128, MCOLS], U16 if fp8 else BF16,
                                 name="mbc")
                mv = mbc[:].rearrange("p (c j e) -> p c j e", c=2, e=MREP)
                if fp8:
                    src0 = m16[:, 0:N].unsqueeze(2).broadcast_to(
                        [128, N, MREP])
                    src1 = m16[:, N:2 * N].unsqueeze(2).broadcast_to(
                        [128, N, MREP])
                else:
                    src0 = g[0][:].unsqueeze(2).broadcast_to([128, N, MREP])
                    src1 = g[1][:].unsqueeze(2).broadcast_to([128, N, MREP])
                nc.scalar.copy(mv[:, 0], src0)
                nc.gpsimd.tensor_copy(mv[:, 1], src1)

                # count[j] = sum_i gate[i, j]
                cnt_ps = ps_cnt.tile([1, N], F32, name="cnt")
                for c in range(2):
                    nc.tensor.matmul(cnt_ps[:], ones[:], g[c][:],
                                     start=(c == 0), stop=(c == 1))
                cnt = gpool.tile([1, N], BF16, name="cnt_sb")
                nc.scalar.copy(cnt[:], cnt_ps[:])

                # ---- stream all 4 batches of edges up front ----------------
                ea_t = []
                for b in range(BL):
                    et = eapool.tile([128, ECOLS], EDT, name="ea_t")
                    nc.sync.dma_start(
                        et[:].rearrange("p (c je) -> p c je", c=2),
                        d_ea[b].rearrange("(c p) je -> p c je", c=2))
                    ea_t.append(et)

                # ---- software-pipelined per-batch stages -------------------
                def stage_head(b):
                    """gate the edge stream, hW, est reduction (PE-heavy)."""
                    et = ea_t[b]
                    if fp8:
                        e16 = et[:].bitcast(U16)
                        nc.vector.tensor_tensor(e16, e16, mbc[:],
                                                AOP.bitwise_and)
                    else:
                        nc.vector.tensor_tensor(et[:], et[:], mbc[:],
                                                AOP.mult)

                    hw_ps = ps_hw.tile([128, 2 * H], F32, name="hw_ps")
                    for c in range(2):
                        nc.tensor.matmul(
                            hw_ps[:, bass.ts(c, H)],
                            hT[:, b * N + 128 * c:b * N + 128 * (c + 1)],
                            whT[:], start=True, stop=True)
                    hw = wpool.tile([128, 2 * H], BF16, name="hw")
                    nc.scalar.copy(hw[:], hw_ps[:])

                    es_ps = ps_es.tile([E, N], F32, name="es_ps")
                    eav = et[:].rearrange("p (c j e) -> p c j e", c=2, e=E)
                    if fp8:
                        selv = sel[:].rearrange("p (e c m) -> p e c m",
                                                c=2, m=E)
                        for e in range(E):
                            nc.tensor.matmul(
                                es_ps[:], selv[:, e], eav[:, :, :, e],
                                start=(e == 0), stop=(e == E - 1),
                                perf_mode=mybir.MatmulPerfMode.DoubleRow)
                    else:
                        for c in range(2):
                            for e in range(E):
                                nc.tensor.matmul(
                                    es_ps[:], sel[:, bass.ts(e, E)],
                                    eav[:, c, :, e],
                                    start=(c == 0 and e == 0),
                                    stop=(c == 1 and e == E - 1))
                    esT = wpool.tile([E, N], BF16, name="esT")
                    nc.scalar.copy(esT[:], es_ps[:])
                    return hw, esT

                def stage_tail(b, hw, esT):
                    """msg accumulation, mask+h, up-projection, store."""
                    msg_ps = ps_msg.tile([H, N], F32, name="msg_ps")
                    for c in range(2):
                        nc.tensor.matmul(msg_ps[:], hw[:, bass.ts(c, H)],
                                         g[c][:], start=(c == 0), stop=False)
                    nc.tensor.matmul(msg_ps[:], wb[:], cnt[:],
                                     start=False, stop=False)
                    nc.tensor.matmul(msg_ps[:], weT[:], esT[:],
                                     start=False, stop=True)

                    xT = wpool.tile([H, N], BF16, name="xT")
                    nc.vector.tensor_tensor(xT[:], msg_ps[:], maskb[b][:],
                                            AOP.mult)
                    nc.vector.tensor_tensor(xT[:], xT[:],
                                            hT[:, bass.ts(b, N)], AOP.add)

                    up_ps = ps_up.tile([H, N], F32, name="up_ps")
                    nc.tensor.matmul(up_ps[:], uwT[:], xT[:],
                                     start=True, stop=True)
                    yt = wpool.tile([H, N], BF16, name="yt")
                    nc.scalar.activation(yt[:], up_ps[:],
                                         mybir.ActivationFunctionType.Identity,
                                         bias=ubc[:])
                    nc.scalar.dma_start(d_y[b], yt[:])

                prev = None
                for b in range(BL):
                    cur = stage_head(b)
                    if prev is not None:
                        stage_tail(b - 1, *prev)
                    prev = cur
                stage_tail(BL - 1, *prev)

    nc.compile()
    return nc


def prep_inputs(h, edge_attr, adj, num_nodes, W_w, W_b, U_w, U_b,
                variant: str = "fp8"):
    """Host-side prep: dtype casts, transposes, adj bit-packing.  Returns a
    dict of full arrays keyed by dram tensor name; index 0 is the shard dim
    for per-core arrays, others are replicated."""
    edt = mybir.dt.np(FP8 if variant == "fp8" else BF16)
    bf = mybir.dt.np(BF16)
    hT = np.ascontiguousarray(
        np.asarray(h, dtype=np.float32).transpose(0, 2, 1)).astype(bf)
    ea = np.asarray(edge_attr, dtype=np.float32).reshape(B, N, NJE).astype(edt)
    adjb4 = np.packbits(np.asarray(adj) != 0, axis=0, bitorder='little')
    adjb = np.ascontiguousarray(adjb4.transpose(1, 2, 0)).view(
        np.uint32)[:, :, 0].astype(np.int32)
    nn = np.asarray(num_nodes).astype(np.int64)
    mask = (np.arange(N)[None, :] < nn[:, None]).astype(bf).reshape(
        N_CORES, 1, BL * N)
    ww = np.asarray(W_w, dtype=np.float32)
    sel_eye = np.eye(E, dtype=np.float32)
    if variant == "fp8":
        # sel[k, 32e + 16c + m] = 1[m == e]
        sel = np.tile(np.stack([sel_eye, sel_eye], axis=1).reshape(1, 32 * E),
                      (128, 1)).astype(edt)
    else:
        sel = np.tile(sel_eye.reshape(1, 16 * E), (128, 1)).astype(edt)
    return {
        "ht": hT, "ea": ea, "adjb": adjb, "mask": mask,
        "whT": np.ascontiguousarray(ww[:, :H].T).astype(bf),
        "weT": np.ascontiguousarray(ww[:, H:].T).astype(bf),
        "uwT": np.ascontiguousarray(np.asarray(U_w, np.float32).T).astype(bf),
        "wb": np.asarray(W_b, np.float32).reshape(1, H).astype(bf),
        "ubc": np.asarray(U_b, np.float32).reshape(H, 1),
        "ones": np.ones((H, 1), dtype=bf),
        "sel": sel,
    }


SHARDED = ("ht", "ea", "mask")


def shard(full, core):
    out = {}
    for k, v in full.items():
        if k in ("ht", "ea"):
            out[k] = v[core * BL:(core + 1) * BL]
        elif k == "mask":
            out[k] = v[core]
        else:
            out[k] = v
    return out


def kernel(h, edge_attr, adj, num_nodes, W_w, W_b, U_w, U_b):
    variant = os.environ.get("KERNEL_VARIANT", "fp8")
    full = prep_inputs(h, edge_attr, adj, num_nodes, W_w, W_b, U_w, U_b,
                       variant)
    nc = build_nc(reps=1, variant=variant)
    in_maps = [shard(full, core) for core in range(N_CORES)]
    res = run_bass_kernel_spmd(nc, in_maps, list(range(N_CORES)))
    out = np.empty((B, N, H), dtype=np.float32)
    for core in range(N_CORES):
        yt = np.asarray(res.results[core]["y"]).astype(np.float32)
        out[core * BL:(core + 1) * BL] = yt.transpose(0, 2, 1)
    return out
